# revision 17
# baseline (speedup 1.0000x reference)
"""Trainium2 kernel for a fuzzy-logic ConjunctionLayer forward pass.

Computes  out = 1[ (1 - x) @ 1[W > 0.5] <= 0 ]  for
x: [8192, 4096] f32, W: [4096, 2048] f32 -> out: [8192, 2048] f32.

Sharding: data-parallel over the batch dim across 8 NeuronCores
(x shard [1024, 4096] per core, W replicated), outputs concatenated.

Math: with x in [0, 1], every term (1-x)*Wb is >= 0, so
  res[m,n] <= 0  <=>  res[m,n] == 0  <=>  no k has (x[m,k] < 1 AND W[k,n] > .5).
The output depends only on the support pattern, so both operands are
binarized on device:
  s  = 1[x < 1],  Wb = 1[W > .5]   (both {0,1}, exact in fp8e4)
  acc = s^T.T @ Wb                 (f32 PSUM accumulation - exact integers)
  out = 1[acc <= 0]
fp8 enables the PE DoubleRow perf mode (2 fp8 weights per cell -> 2x
MACs/cycle, contraction 256 per matmul).

The kernel is DMA-bound (56 MB/core), so the schedule maximizes HBM
bandwidth: every transfer is >= 0.5 MB with >= 4 KB contiguous
per-partition rows (the host pre-permutes x and W into k-pair-major
layouts to make that possible), transfers alternate across the two HWDGE
rings (Sync + Scalar issue queues), and the 8 batch-chunk accumulation
chains (one PSUM bank each) consume every k-pair the moment it lands.
W streams n-block-major so each 8 MB block is reused by all 8 chains
while the next block prefetches; output stores are split between the
GPSIMD SWDGE queue and the rings.
"""

import numpy as np

import concourse.bass as bass
import concourse.mybir as mybir
import concourse.tile as tile
from concourse import bacc
from concourse.bass_utils import run_bass_kernel_spmd

BATCH, IN_DIM, N_RULES = 8192, 4096, 2048
N_CORES = 8
M_LOCAL = BATCH // N_CORES  # 1024 batch rows per core

P = 128            # SBUF partitions / matmul tile edge
NB_W = 512         # n-block width (= one f32 PSUM bank)
NB = N_RULES // NB_W        # 4 n-blocks
KT = IN_DIM // P            # 32 k-tiles
KP = KT // 2                # 16 k-pairs (DoubleRow consumes 2 per matmul)
MT = M_LOCAL // P           # 8 batch chunks per core

F32 = mybir.dt.float32
BF16 = mybir.dt.bfloat16
FP8 = mybir.dt.float8e4
ALU = mybir.AluOpType
DR = mybir.MatmulPerfMode.DoubleRow


def _body(tc: tile.TileContext, out: bass.AP, xp: bass.AP, wp: bass.AP):
    nc = tc.nc
    rings = (nc.sync, nc.scalar)  # the two HWDGE issue queues
    with (
        tc.tile_pool(name="sb", bufs=1) as sb,
        tc.tile_pool(name="ps", bufs=1, space="PSUM") as ps,
    ):
        # Resident binarized operands (2D tiles; matmul slices them as
        # [128, 2, .] k-pair APs via rearrange).
        s2 = [sb.tile([P, 2 * M_LOCAL], FP8, tag=f"s{kk}", bufs=1,
                      name=f"s{kk}") for kk in range(KP)]
        wb2 = [[sb.tile([P, 2 * NB_W], FP8, tag=f"wb{nb}_{kk}", bufs=1,
                        name=f"wb{nb}_{kk}") for kk in range(KP)]
               for nb in range(NB)]
        def load_x_pair(kk):
            # two half-slab DMAs on opposite rings; deep bufs so the rings
            # always have queued transfers (a drained ring wastes HBM BW)
            for j in (0, 1):
                xf = sb.tile([P, M_LOCAL], F32, tag="xf", bufs=8,
                             name=f"xf{kk}_{j}")
                rings[(kk + j) % 2].dma_start(xf[:], xp[kk, :,
                                                        j * M_LOCAL:
                                                        (j + 1) * M_LOCAL])
                nc.vector.tensor_scalar(s2[kk][:, j * M_LOCAL:
                                               (j + 1) * M_LOCAL],
                                        xf[:], 1.0, None, ALU.is_lt)

        def load_w_pair(nb, kk):
            # halved across both rings for per-k-pair balance
            wf = sb.tile([P, 2 * NB_W], F32, tag="wf", bufs=8,
                         name=f"wf{nb}_{kk}")
            for j in (0, 1):
                rings[(kk + j) % 2].dma_start(
                    wf[:, j * NB_W:(j + 1) * NB_W],
                    wp[kk * NB + nb, :, j * NB_W:(j + 1) * NB_W])
            nc.vector.tensor_scalar(wb2[nb][kk][:], wf[:], 0.5, None,
                                    ALU.is_gt)

        # Global W-load pump: emits W transfers in consumption order with a
        # fixed prefetch lead so the rings never drain at phase boundaries.
        w_order = [(nb, kk) for nb in range(NB) for kk in range(KP)]
        w_state = {"next": 0}

        def pump_w(consumed, lead):
            target = min(len(w_order), consumed + 1 + lead)
            while w_state["next"] < target:
                nb, kk = w_order[w_state["next"]]
                load_w_pair(nb, kk)
                w_state["next"] += 1

        accs = {}

        def mm_step(nb, kk):
            """All 8 batch chains consume k-pair kk of n-block nb."""
            rhs = wb2[nb][kk][:].rearrange("p (two n) -> p two n", two=2)
            lhsT = s2[kk][:].rearrange("p (two m) -> p two m", two=2)
            for m in range(MT):
                if kk == 0:
                    accs[m] = ps.tile([P, NB_W], F32, tag=f"acc{m}", bufs=1,
                                      name=f"acc{nb}_{m}")
                nc.tensor.matmul(
                    accs[m][:],
                    lhsT[:, :, m * P:(m + 1) * P],
                    rhs,
                    start=(kk == 0),
                    stop=(kk == KP - 1),
                    perf_mode=DR,
                )

        def epilogue(nb):
            # bf16 stores: {0,1} is exact in bf16 and halves write traffic;
            # the host widens back to f32.
            for m in range(MT):
                o = sb.tile([P, NB_W], BF16, tag="o", bufs=6,
                            name=f"o{nb}_{m}")
                nc.vector.tensor_scalar(o[:], accs[m][:], 0.0, None,
                                        ALU.is_le)
                eng = nc.gpsimd if m % 2 == 0 else rings[(nb + m // 2) % 2]
                eng.dma_start(
                    out[m * P:(m + 1) * P, nb * NB_W:(nb + 1) * NB_W],
                    o[:])

        # n-block 0: stream x + W k-pair-wise so chains ride the DMA.
        # During phase A the pump stays at lead 1 (phase A is DMA-bound;
        # deeper W prefetch would delay the x/W the chains need now) and
        # ramps to full lead near the end so nb1 data is in flight when
        # phase A's last chains retire.
        for kk in range(KP):
            load_x_pair(kk)
            pump_w(kk, lead=max(0, 2 * (kk - (KP - 4))))
            mm_step(0, kk)
        epilogue(0)

        # n-blocks 1..3: W streams with prefetch lead, chains consume on
        # arrival
        for nb in range(1, NB):
            for kk in range(KP):
                pump_w(nb * KP + kk, lead=6)
                mm_step(nb, kk)
            epilogue(nb)


_NC_CACHE = {}


def _get_nc():
    if "nc" not in _NC_CACHE:
        nc = bacc.Bacc("TRN2", target_bir_lowering=False, debug=False,
                       num_devices=N_CORES)
        xp = nc.dram_tensor("xp", [KP, P, 2 * M_LOCAL], F32,
                            kind="ExternalInput")
        wp = nc.dram_tensor("wp", [KP * NB, P, 2 * NB_W], F32,
                            kind="ExternalInput")
        out = nc.dram_tensor("out", [M_LOCAL, N_RULES], BF16,
                             kind="ExternalOutput")
        with tile.TileContext(nc) as tc:
            _body(tc, out.ap(), xp.ap(), wp.ap())
        nc.compile()
        _NC_CACHE["nc"] = nc
    return _NC_CACHE["nc"]


def _permute_w(W: np.ndarray) -> np.ndarray:
    # [IN_DIM, N_RULES] -> [KP*NB, P, 2*NB_W]: for each k-pair kk and
    # n-block nb, row p holds [W[2kk*128+p, nb-block], W[(2kk+1)*128+p, ...]]
    w5 = W.reshape(KP, 2, P, NB, NB_W)          # [kk, j, p, nb, n]
    return np.ascontiguousarray(
        w5.transpose(0, 3, 2, 1, 4).reshape(KP * NB, P, 2 * NB_W))


def _permute_x(x_shard: np.ndarray) -> np.ndarray:
    # [M_LOCAL, IN_DIM] -> [KP, P, 2*M_LOCAL]: row p of slab kk holds
    # [x[:, 2kk*128+p].T, x[:, (2kk+1)*128+p].T]
    x4 = x_shard.T.reshape(KP, 2, P, M_LOCAL)   # [kk, j, p, m]
    return np.ascontiguousarray(x4.transpose(0, 2, 1, 3).reshape(
        KP, P, 2 * M_LOCAL))


def kernel(x: np.ndarray, W: np.ndarray, **run_kwargs) -> np.ndarray:
    assert x.shape == (BATCH, IN_DIM) and W.shape == (IN_DIM, N_RULES)
    x = np.ascontiguousarray(x, dtype=np.float32)
    W = np.ascontiguousarray(W, dtype=np.float32)
    nc = _get_nc()
    wp = _permute_w(W)
    in_maps = []
    for c in range(N_CORES):
        in_maps.append({"xp": _permute_x(x[c * M_LOCAL:(c + 1) * M_LOCAL, :]),
                        "wp": wp})
    res = run_bass_kernel_spmd(nc, in_maps, core_ids=list(range(N_CORES)),
                               **run_kwargs)
    out = np.concatenate([res.results[c]["out"] for c in range(N_CORES)],
                         axis=0).astype(np.float32)  # bf16 {0,1} -> f32 exact
    if run_kwargs:
        kernel.last_results = res
    return out


# revision 20
# speedup vs baseline: 1.1090x; 1.1090x over previous
"""Trainium2 kernel for a fuzzy-logic ConjunctionLayer forward pass.

Computes  out = 1[ (1 - x) @ 1[W > 0.5] <= 0 ]  for
x: [8192, 4096] f32, W: [4096, 2048] f32 -> out: [8192, 2048] f32.

Sharding: data-parallel over the batch dim across 8 NeuronCores
(x shard [1024, 4096] per core, W replicated), outputs concatenated.

Math: with x in [0, 1], every term (1-x)*Wb is >= 0, so
  res[m,n] <= 0  <=>  res[m,n] == 0  <=>  no k has (x[m,k] < 1 AND W[k,n] > .5).
The output depends only on the support pattern, so both operands are
binarized on device:
  s  = 1[x < 1],  Wb = 1[W > .5]   (both {0,1}, exact in fp8e4)
  acc = s^T.T @ Wb                 (f32 PSUM accumulation - exact integers)
  out = 1[acc <= 0]
fp8 enables the PE DoubleRow perf mode (2 fp8 weights per cell -> 2x
MACs/cycle, contraction 256 per matmul).

The kernel is DMA-bound (56 MB/core), so the schedule maximizes HBM
bandwidth: every transfer is >= 0.5 MB with >= 4 KB contiguous
per-partition rows (the host pre-permutes x and W into k-pair-major
layouts to make that possible), transfers alternate across the two HWDGE
rings (Sync + Scalar issue queues), and the 8 batch-chunk accumulation
chains (one PSUM bank each) consume every k-pair the moment it lands.
W streams n-block-major so each 8 MB block is reused by all 8 chains
while the next block prefetches; output stores are split between the
GPSIMD SWDGE queue and the rings.
"""

import numpy as np

import concourse.bass as bass
import concourse.mybir as mybir
import concourse.tile as tile
from concourse import bacc
from concourse.bass_utils import run_bass_kernel_spmd

BATCH, IN_DIM, N_RULES = 8192, 4096, 2048
N_CORES = 8
M_LOCAL = BATCH // N_CORES  # 1024 batch rows per core

P = 128            # SBUF partitions / matmul tile edge
NB_W = 512         # n-block width (= one f32 PSUM bank)
NB = N_RULES // NB_W        # 4 n-blocks
KT = IN_DIM // P            # 32 k-tiles
KP = KT // 2                # 16 k-pairs (DoubleRow consumes 2 per matmul)
MT = M_LOCAL // P           # 8 batch chunks per core

F32 = mybir.dt.float32
BF16 = mybir.dt.bfloat16
FP8 = mybir.dt.float8e4
ALU = mybir.AluOpType
DR = mybir.MatmulPerfMode.DoubleRow


def _body(tc: tile.TileContext, out: bass.AP, xp: bass.AP, wp: bass.AP):
    nc = tc.nc
    rings = (nc.sync, nc.scalar)  # the two HWDGE issue queues
    with (
        tc.tile_pool(name="sb", bufs=1) as sb,
        tc.tile_pool(name="ps", bufs=1, space="PSUM") as ps,
    ):
        # Resident binarized operands (2D tiles; matmul slices them as
        # [128, 2, .] k-pair APs via rearrange).
        s2 = [sb.tile([P, 2 * M_LOCAL], FP8, tag=f"s{kk}", bufs=1,
                      name=f"s{kk}") for kk in range(KP)]
        wb2 = [[sb.tile([P, 2 * NB_W], FP8, tag=f"wb{nb}_{kk}", bufs=1,
                        name=f"wb{nb}_{kk}") for kk in range(KP)]
               for nb in range(NB)]
        def load_x_pair(kk):
            # two half-slab DMAs on opposite rings; deep bufs so the rings
            # always have queued transfers (a drained ring wastes HBM BW)
            for j in (0, 1):
                xf = sb.tile([P, M_LOCAL], F32, tag="xf", bufs=8,
                             name=f"xf{kk}_{j}")
                rings[(kk + j) % 2].dma_start(xf[:], xp[kk, :,
                                                        j * M_LOCAL:
                                                        (j + 1) * M_LOCAL])
                nc.vector.tensor_scalar(s2[kk][:, j * M_LOCAL:
                                               (j + 1) * M_LOCAL],
                                        xf[:], 1.0, None, ALU.is_lt)

        def load_w_pair(nb, kk, split):
            wf = sb.tile([P, 2 * NB_W], F32, tag="wf", bufs=8,
                         name=f"wf{nb}_{kk}")
            if split:
                # halved across both rings for per-k-pair balance (phase A)
                for j in (0, 1):
                    rings[(kk + j) % 2].dma_start(
                        wf[:, j * NB_W:(j + 1) * NB_W],
                        wp[kk * NB + nb, :, j * NB_W:(j + 1) * NB_W])
            else:
                # one 4 KB-row transfer, rings alternate by k-pair
                rings[kk % 2].dma_start(wf[:], wp[kk * NB + nb])
            nc.vector.tensor_scalar(wb2[nb][kk][:], wf[:], 0.5, None,
                                    ALU.is_gt)

        # Global W-load pump: emits W transfers in consumption order with a
        # fixed prefetch lead so the rings never drain at phase boundaries.
        w_order = [(nb, kk) for nb in range(NB) for kk in range(KP)]
        w_state = {"next": 0}

        def pump_w(consumed, lead):
            target = min(len(w_order), consumed + 1 + lead)
            while w_state["next"] < target:
                nb, kk = w_order[w_state["next"]]
                load_w_pair(nb, kk, split=(nb == 0))
                w_state["next"] += 1

        accs = {}

        def mm_step(nb, kk):
            """All 8 batch chains consume k-pair kk of n-block nb."""
            rhs = wb2[nb][kk][:].rearrange("p (two n) -> p two n", two=2)
            lhsT = s2[kk][:].rearrange("p (two m) -> p two m", two=2)
            for m in range(MT):
                if kk == 0:
                    accs[m] = ps.tile([P, NB_W], F32, tag=f"acc{m}", bufs=1,
                                      name=f"acc{nb}_{m}")
                nc.tensor.matmul(
                    accs[m][:],
                    lhsT[:, :, m * P:(m + 1) * P],
                    rhs,
                    start=(kk == 0),
                    stop=(kk == KP - 1),
                    perf_mode=DR,
                )

        def epilogue(nb):
            # bf16 stores: {0,1} is exact in bf16 and halves write traffic;
            # the host widens back to f32.
            for m in range(MT):
                o = sb.tile([P, NB_W], BF16, tag="o", bufs=6,
                            name=f"o{nb}_{m}")
                nc.vector.tensor_scalar(o[:], accs[m][:], 0.0, None,
                                        ALU.is_le)
                eng = nc.gpsimd if m % 2 == 0 else rings[(nb + m // 2) % 2]
                eng.dma_start(
                    out[m * P:(m + 1) * P, nb * NB_W:(nb + 1) * NB_W],
                    o[:])

        # n-block 0: stream x + W k-pair-wise so chains ride the DMA.
        # During phase A the pump stays at lead 1 (phase A is DMA-bound;
        # deeper W prefetch would delay the x/W the chains need now) and
        # ramps to full lead near the end so nb1 data is in flight when
        # phase A's last chains retire.
        for kk in range(KP):
            load_x_pair(kk)
            pump_w(kk, lead=0)
            mm_step(0, kk)
        # all of x is emitted - queue nb1's first k-pairs behind it so the
        # rings don't drain at the phase boundary
        pump_w(KP - 1, lead=6)
        epilogue(0)

        # n-blocks 1..3: W streams with prefetch lead, chains consume on
        # arrival
        for nb in range(1, NB):
            for kk in range(KP):
                pump_w(nb * KP + kk, lead=6)
                mm_step(nb, kk)
            epilogue(nb)


_NC_CACHE = {}


def _get_nc():
    if "nc" not in _NC_CACHE:
        nc = bacc.Bacc("TRN2", target_bir_lowering=False, debug=False,
                       num_devices=N_CORES)
        xp = nc.dram_tensor("xp", [KP, P, 2 * M_LOCAL], F32,
                            kind="ExternalInput")
        wp = nc.dram_tensor("wp", [KP * NB, P, 2 * NB_W], F32,
                            kind="ExternalInput")
        out = nc.dram_tensor("out", [M_LOCAL, N_RULES], BF16,
                             kind="ExternalOutput")
        with tile.TileContext(nc) as tc:
            _body(tc, out.ap(), xp.ap(), wp.ap())
        nc.compile()
        _NC_CACHE["nc"] = nc
    return _NC_CACHE["nc"]


def _permute_w(W: np.ndarray) -> np.ndarray:
    # [IN_DIM, N_RULES] -> [KP*NB, P, 2*NB_W]: for each k-pair kk and
    # n-block nb, row p holds [W[2kk*128+p, nb-block], W[(2kk+1)*128+p, ...]]
    w5 = W.reshape(KP, 2, P, NB, NB_W)          # [kk, j, p, nb, n]
    return np.ascontiguousarray(
        w5.transpose(0, 3, 2, 1, 4).reshape(KP * NB, P, 2 * NB_W))


def _permute_x(x_shard: np.ndarray) -> np.ndarray:
    # [M_LOCAL, IN_DIM] -> [KP, P, 2*M_LOCAL]: row p of slab kk holds
    # [x[:, 2kk*128+p].T, x[:, (2kk+1)*128+p].T]
    x4 = x_shard.T.reshape(KP, 2, P, M_LOCAL)   # [kk, j, p, m]
    return np.ascontiguousarray(x4.transpose(0, 2, 1, 3).reshape(
        KP, P, 2 * M_LOCAL))


def kernel(x: np.ndarray, W: np.ndarray, **run_kwargs) -> np.ndarray:
    assert x.shape == (BATCH, IN_DIM) and W.shape == (IN_DIM, N_RULES)
    x = np.ascontiguousarray(x, dtype=np.float32)
    W = np.ascontiguousarray(W, dtype=np.float32)
    nc = _get_nc()
    wp = _permute_w(W)
    in_maps = []
    for c in range(N_CORES):
        in_maps.append({"xp": _permute_x(x[c * M_LOCAL:(c + 1) * M_LOCAL, :]),
                        "wp": wp})
    res = run_bass_kernel_spmd(nc, in_maps, core_ids=list(range(N_CORES)),
                               **run_kwargs)
    out = np.concatenate([res.results[c]["out"] for c in range(N_CORES)],
                         axis=0).astype(np.float32)  # bf16 {0,1} -> f32 exact
    if run_kwargs:
        kernel.last_results = res
    return out


# revision 22
# speedup vs baseline: 1.1256x; 1.0150x over previous
"""Trainium2 kernel for a fuzzy-logic ConjunctionLayer forward pass.

Computes  out = 1[ (1 - x) @ 1[W > 0.5] <= 0 ]  for
x: [8192, 4096] f32, W: [4096, 2048] f32 -> out: [8192, 2048] f32.

Sharding: data-parallel over the batch dim across 8 NeuronCores
(x shard [1024, 4096] per core, W replicated), outputs concatenated.

Math: with x in [0, 1], every term (1-x)*Wb is >= 0, so
  res[m,n] <= 0  <=>  res[m,n] == 0  <=>  no k has (x[m,k] < 1 AND W[k,n] > .5).
The output depends only on the support pattern, so both operands are
binarized on device:
  s  = 1[x < 1],  Wb = 1[W > .5]   (both {0,1}, exact in fp8e4)
  acc = s^T.T @ Wb                 (f32 PSUM accumulation - exact integers)
  out = 1[acc <= 0]
fp8 enables the PE DoubleRow perf mode (2 fp8 weights per cell -> 2x
MACs/cycle, contraction 256 per matmul).

The kernel is DMA-bound (56 MB/core), so the schedule maximizes HBM
bandwidth: every transfer is >= 0.5 MB with >= 4 KB contiguous
per-partition rows (the host pre-permutes x and W into k-pair-major
layouts to make that possible), transfers alternate across the two HWDGE
rings (Sync + Scalar issue queues), and the 8 batch-chunk accumulation
chains (one PSUM bank each) consume every k-pair the moment it lands.
W streams n-block-major so each 8 MB block is reused by all 8 chains
while the next block prefetches; output stores are split between the
GPSIMD SWDGE queue and the rings.
"""

import numpy as np

import concourse.bass as bass
import concourse.mybir as mybir
import concourse.tile as tile
from concourse import bacc
from concourse.bass_utils import run_bass_kernel_spmd

BATCH, IN_DIM, N_RULES = 8192, 4096, 2048
N_CORES = 8
M_LOCAL = BATCH // N_CORES  # 1024 batch rows per core

P = 128            # SBUF partitions / matmul tile edge
NB_W = 512         # n-block width (= one f32 PSUM bank)
NB = N_RULES // NB_W        # 4 n-blocks
KT = IN_DIM // P            # 32 k-tiles
KP = KT // 2                # 16 k-pairs (DoubleRow consumes 2 per matmul)
MT = M_LOCAL // P           # 8 batch chunks per core

F32 = mybir.dt.float32
BF16 = mybir.dt.bfloat16
FP8 = mybir.dt.float8e4
ALU = mybir.AluOpType
DR = mybir.MatmulPerfMode.DoubleRow


def _body(tc: tile.TileContext, out: bass.AP, xp: bass.AP, wp: bass.AP):
    nc = tc.nc
    rings = (nc.sync, nc.scalar)  # the two HWDGE issue queues
    with (
        tc.tile_pool(name="sb", bufs=1) as sb,
        tc.tile_pool(name="ps", bufs=1, space="PSUM") as ps,
    ):
        # Resident binarized operands (2D tiles; matmul slices them as
        # [128, 2, .] k-pair APs via rearrange).
        s2 = [sb.tile([P, 2 * M_LOCAL], FP8, tag=f"s{kk}", bufs=1,
                      name=f"s{kk}") for kk in range(KP)]
        wb2 = [[sb.tile([P, 2 * NB_W], FP8, tag=f"wb{nb}_{kk}", bufs=1,
                        name=f"wb{nb}_{kk}") for kk in range(KP)]
               for nb in range(NB)]
        def load_x_pair(kk):
            # two half-slab DMAs on opposite rings; deep bufs so the rings
            # always have queued transfers (a drained ring wastes HBM BW)
            for j in (0, 1):
                xf = sb.tile([P, M_LOCAL], F32, tag="xf", bufs=8,
                             name=f"xf{kk}_{j}")
                rings[(kk + j) % 2].dma_start(xf[:], xp[kk, :,
                                                        j * M_LOCAL:
                                                        (j + 1) * M_LOCAL])
                nc.vector.tensor_scalar(s2[kk][:, j * M_LOCAL:
                                               (j + 1) * M_LOCAL],
                                        xf[:], 1.0, None, ALU.is_lt)

        def load_w_pair(nb, kk, split):
            wf = sb.tile([P, 2 * NB_W], F32, tag="wf", bufs=8,
                         name=f"wf{nb}_{kk}")
            if split:
                # halved across both rings for per-k-pair balance (phase A)
                for j in (0, 1):
                    rings[(kk + j) % 2].dma_start(
                        wf[:, j * NB_W:(j + 1) * NB_W],
                        wp[kk * NB + nb, :, j * NB_W:(j + 1) * NB_W])
            else:
                # one 4 KB-row transfer, rings alternate by k-pair
                rings[kk % 2].dma_start(wf[:], wp[kk * NB + nb])
            nc.vector.tensor_scalar(wb2[nb][kk][:], wf[:], 0.5, None,
                                    ALU.is_gt)

        # Global W-load pump: emits W transfers in consumption order with a
        # fixed prefetch lead so the rings never drain at phase boundaries.
        w_order = [(nb, kk) for nb in range(NB) for kk in range(KP)]
        w_state = {"next": 0}

        def pump_w(consumed, lead):
            target = min(len(w_order), consumed + 1 + lead)
            while w_state["next"] < target:
                nb, kk = w_order[w_state["next"]]
                load_w_pair(nb, kk, split=(nb == 0))
                w_state["next"] += 1

        accs = {}

        def epilogue_m(nb, m):
            # bf16 stores: {0,1} is exact in bf16 and halves write traffic;
            # the host widens back to f32. Emitted right after chain m's
            # stop-matmul so its PSUM bank frees while later chains finish.
            o = sb.tile([P, NB_W], BF16, tag="o", bufs=6, name=f"o{nb}_{m}")
            nc.vector.tensor_scalar(o[:], accs[m][:], 0.0, None, ALU.is_le)
            eng = nc.gpsimd if m % 2 == 0 else rings[(nb + m // 2) % 2]
            eng.dma_start(
                out[m * P:(m + 1) * P, nb * NB_W:(nb + 1) * NB_W], o[:])

        def mm_step(nb, kk):
            """All 8 batch chains consume k-pair kk of n-block nb."""
            rhs = wb2[nb][kk][:].rearrange("p (two n) -> p two n", two=2)
            lhsT = s2[kk][:].rearrange("p (two m) -> p two m", two=2)
            for m in range(MT):
                if kk == 0:
                    accs[m] = ps.tile([P, NB_W], F32, tag=f"acc{m}", bufs=1,
                                      name=f"acc{nb}_{m}")
                nc.tensor.matmul(
                    accs[m][:],
                    lhsT[:, :, m * P:(m + 1) * P],
                    rhs,
                    start=(kk == 0),
                    stop=(kk == KP - 1),
                    perf_mode=DR,
                )
                if kk == KP - 1:
                    epilogue_m(nb, m)

        # n-block 0: stream x + W k-pair-wise so chains ride the DMA.
        # During phase A the pump stays at lead 1 (phase A is DMA-bound;
        # deeper W prefetch would delay the x/W the chains need now) and
        # ramps to full lead near the end so nb1 data is in flight when
        # phase A's last chains retire.
        for kk in range(KP):
            load_x_pair(kk)
            if kk == KP - 1:
                # all of x is emitted - queue nb1's first k-pairs behind it
                # so the rings don't drain at the phase boundary
                pump_w(KP - 1, lead=6)
            else:
                pump_w(kk, lead=0)
            mm_step(0, kk)

        # n-blocks 1..3: W streams with prefetch lead, chains consume on
        # arrival
        for nb in range(1, NB):
            for kk in range(KP):
                pump_w(nb * KP + kk, lead=6)
                mm_step(nb, kk)


_NC_CACHE = {}


def _get_nc():
    if "nc" not in _NC_CACHE:
        nc = bacc.Bacc("TRN2", target_bir_lowering=False, debug=False,
                       num_devices=N_CORES)
        xp = nc.dram_tensor("xp", [KP, P, 2 * M_LOCAL], F32,
                            kind="ExternalInput")
        wp = nc.dram_tensor("wp", [KP * NB, P, 2 * NB_W], F32,
                            kind="ExternalInput")
        out = nc.dram_tensor("out", [M_LOCAL, N_RULES], BF16,
                             kind="ExternalOutput")
        with tile.TileContext(nc) as tc:
            _body(tc, out.ap(), xp.ap(), wp.ap())
        nc.compile()
        _NC_CACHE["nc"] = nc
    return _NC_CACHE["nc"]


def _permute_w(W: np.ndarray) -> np.ndarray:
    # [IN_DIM, N_RULES] -> [KP*NB, P, 2*NB_W]: for each k-pair kk and
    # n-block nb, row p holds [W[2kk*128+p, nb-block], W[(2kk+1)*128+p, ...]]
    w5 = W.reshape(KP, 2, P, NB, NB_W)          # [kk, j, p, nb, n]
    return np.ascontiguousarray(
        w5.transpose(0, 3, 2, 1, 4).reshape(KP * NB, P, 2 * NB_W))


def _permute_x(x_shard: np.ndarray) -> np.ndarray:
    # [M_LOCAL, IN_DIM] -> [KP, P, 2*M_LOCAL]: row p of slab kk holds
    # [x[:, 2kk*128+p].T, x[:, (2kk+1)*128+p].T]
    x4 = x_shard.T.reshape(KP, 2, P, M_LOCAL)   # [kk, j, p, m]
    return np.ascontiguousarray(x4.transpose(0, 2, 1, 3).reshape(
        KP, P, 2 * M_LOCAL))


def kernel(x: np.ndarray, W: np.ndarray, **run_kwargs) -> np.ndarray:
    assert x.shape == (BATCH, IN_DIM) and W.shape == (IN_DIM, N_RULES)
    x = np.ascontiguousarray(x, dtype=np.float32)
    W = np.ascontiguousarray(W, dtype=np.float32)
    nc = _get_nc()
    wp = _permute_w(W)
    in_maps = []
    for c in range(N_CORES):
        in_maps.append({"xp": _permute_x(x[c * M_LOCAL:(c + 1) * M_LOCAL, :]),
                        "wp": wp})
    res = run_bass_kernel_spmd(nc, in_maps, core_ids=list(range(N_CORES)),
                               **run_kwargs)
    out = np.concatenate([res.results[c]["out"] for c in range(N_CORES)],
                         axis=0).astype(np.float32)  # bf16 {0,1} -> f32 exact
    if run_kwargs:
        kernel.last_results = res
    return out


# revision 23
# speedup vs baseline: 1.4952x; 1.3283x over previous
"""Trainium2 kernel for a fuzzy-logic ConjunctionLayer forward pass.

Computes  out = 1[ (1 - x) @ 1[W > 0.5] <= 0 ]  for
x: [8192, 4096] f32, W: [4096, 2048] f32 -> out: [8192, 2048] f32.

Sharding: data-parallel over the batch dim across 8 NeuronCores
(x shard [1024, 4096] per core, W replicated), outputs concatenated.

Math: with x in [0, 1], every term (1-x)*Wb is >= 0, so
  res[m,n] <= 0  <=>  res[m,n] == 0  <=>  no k has (x[m,k] < 1 AND W[k,n] > .5).
The output depends only on the support pattern, so both operands are
binarized on device:
  s  = 1[x < 1],  Wb = 1[W > .5]   (both {0,1}, exact in fp8e4)
  acc = s^T.T @ Wb                 (f32 PSUM accumulation - exact integers)
  out = 1[acc <= 0]
fp8 enables the PE DoubleRow perf mode (2 fp8 weights per cell -> 2x
MACs/cycle, contraction 256 per matmul).

Transport encodings (all lossless FOR THE PREDICATES, on the whole real
line, proven + property-tested):
  - x ships as round-toward-zero f16: truncation is monotone with 1.0 a
    fixed point, so rtz(x) < 1  <=>  x < 1 exactly.
  - W ships as round-toward-+inf f16: 0.5 is representable, so
    rup(W) > 0.5  <=>  W > 0.5 exactly.
  - out ships as bf16 ({0,1} exact), widened to f32 on the host.
This halves input DMA (48 -> 24 MB/core), turning a DMA-bound kernel
into a PE-bound one. The device still performs all thresholding,
matmuls, and output classification.

Schedule: N is split into four 512-wide blocks (one f32 PSUM bank per
batch-chunk chain, 8 chains in flight). Phase A streams x slabs + the
first W block k-pair-wise across the two HWDGE rings (Sync/Scalar) so
chains ride the DMA; later W blocks prefetch with a fixed lead through
a global pump so rings never drain at phase boundaries. Each chain's
threshold epilogue is emitted right after its stop-matmul so its PSUM
bank frees immediately. Output stores split between GPSIMD SWDGE and
the rings. The host pre-permutes x and W into k-pair-major layouts so
every transfer has >= 2 KB contiguous per-partition rows.
"""

import numpy as np

import concourse.bass as bass
import concourse.mybir as mybir
import concourse.tile as tile
from concourse import bacc
from concourse.bass_utils import run_bass_kernel_spmd

BATCH, IN_DIM, N_RULES = 8192, 4096, 2048
N_CORES = 8
M_LOCAL = BATCH // N_CORES  # 1024 batch rows per core

P = 128            # SBUF partitions / matmul tile edge
NB_W = 512         # n-block width (= one f32 PSUM bank)
NB = N_RULES // NB_W        # 4 n-blocks
KT = IN_DIM // P            # 32 k-tiles
KP = KT // 2                # 16 k-pairs (DoubleRow consumes 2 per matmul)
MT = M_LOCAL // P           # 8 batch chunks per core

F32 = mybir.dt.float32
F16 = mybir.dt.float16
BF16 = mybir.dt.bfloat16
FP8 = mybir.dt.float8e4
ALU = mybir.AluOpType
DR = mybir.MatmulPerfMode.DoubleRow


def _body(tc: tile.TileContext, out: bass.AP, xp: bass.AP, wp: bass.AP):
    nc = tc.nc
    rings = (nc.sync, nc.scalar)  # the two HWDGE issue queues
    with (
        tc.tile_pool(name="sb", bufs=1) as sb,
        tc.tile_pool(name="ps", bufs=1, space="PSUM") as ps,
    ):
        # Resident binarized operands (2D tiles; matmul slices them as
        # [128, 2, .] k-pair APs via rearrange).
        s2 = [sb.tile([P, 2 * M_LOCAL], FP8, tag=f"s{kk}", bufs=1,
                      name=f"s{kk}") for kk in range(KP)]
        wb2 = [[sb.tile([P, 2 * NB_W], FP8, tag=f"wb{nb}_{kk}", bufs=1,
                        name=f"wb{nb}_{kk}") for kk in range(KP)]
               for nb in range(NB)]

        def load_x_pair(kk):
            xf = sb.tile([P, 2 * M_LOCAL], F16, tag="xf", bufs=6,
                         name=f"xf{kk}")
            rings[kk % 2].dma_start(xf[:], xp[kk])
            nc.vector.tensor_scalar(s2[kk][:], xf[:], 1.0, None, ALU.is_lt)

        def load_w_pair(nb, kk):
            wf = sb.tile([P, 2 * NB_W], F16, tag="wf", bufs=8,
                         name=f"wf{nb}_{kk}")
            rings[(kk + 1) % 2].dma_start(wf[:], wp[kk * NB + nb])
            nc.vector.tensor_scalar(wb2[nb][kk][:], wf[:], 0.5, None,
                                    ALU.is_gt)

        # Global W-load pump: emits W transfers in consumption order with a
        # prefetch lead so the rings never drain at phase boundaries.
        w_order = [(nb, kk) for nb in range(NB) for kk in range(KP)]
        w_state = {"next": 0}

        def pump_w(consumed, lead):
            target = min(len(w_order), consumed + 1 + lead)
            while w_state["next"] < target:
                nb, kk = w_order[w_state["next"]]
                load_w_pair(nb, kk)
                w_state["next"] += 1

        accs = {}

        def epilogue_m(nb, m):
            # bf16 stores: {0,1} exact, half the write traffic; emitted
            # right after chain m's stop-matmul so its PSUM bank frees
            # while later chains finish.
            o = sb.tile([P, NB_W], BF16, tag="o", bufs=6, name=f"o{nb}_{m}")
            nc.vector.tensor_scalar(o[:], accs[m][:], 0.0, None, ALU.is_le)
            eng = nc.gpsimd if m % 2 == 0 else rings[(nb + m // 2) % 2]
            eng.dma_start(
                out[m * P:(m + 1) * P, nb * NB_W:(nb + 1) * NB_W], o[:])

        def mm_step(nb, kk):
            """All 8 batch chains consume k-pair kk of n-block nb."""
            rhs = wb2[nb][kk][:].rearrange("p (two n) -> p two n", two=2)
            lhsT = s2[kk][:].rearrange("p (two m) -> p two m", two=2)
            for m in range(MT):
                if kk == 0:
                    accs[m] = ps.tile([P, NB_W], F32, tag=f"acc{m}", bufs=1,
                                      name=f"acc{nb}_{m}")
                nc.tensor.matmul(
                    accs[m][:],
                    lhsT[:, :, m * P:(m + 1) * P],
                    rhs,
                    start=(kk == 0),
                    stop=(kk == KP - 1),
                    perf_mode=DR,
                )
                if kk == KP - 1:
                    epilogue_m(nb, m)

        # n-block 0: stream x + W k-pair-wise so chains ride the DMA
        for kk in range(KP):
            load_x_pair(kk)
            if kk == KP - 1:
                # all of x is emitted - queue nb1's first k-pairs behind it
                # so the rings don't drain at the phase boundary
                pump_w(KP - 1, lead=6)
            else:
                pump_w(kk, lead=0)
            mm_step(0, kk)

        # n-blocks 1..3: W streams with prefetch lead, chains consume on
        # arrival
        for nb in range(1, NB):
            for kk in range(KP):
                pump_w(nb * KP + kk, lead=6)
                mm_step(nb, kk)


_NC_CACHE = {}


def _get_nc():
    if "nc" not in _NC_CACHE:
        nc = bacc.Bacc("TRN2", target_bir_lowering=False, debug=False,
                       num_devices=N_CORES)
        xp = nc.dram_tensor("xp", [KP, P, 2 * M_LOCAL], F16,
                            kind="ExternalInput")
        wp = nc.dram_tensor("wp", [KP * NB, P, 2 * NB_W], F16,
                            kind="ExternalInput")
        out = nc.dram_tensor("out", [M_LOCAL, N_RULES], BF16,
                             kind="ExternalOutput")
        with tile.TileContext(nc) as tc:
            _body(tc, out.ap(), xp.ap(), wp.ap())
        nc.compile()
        _NC_CACHE["nc"] = nc
    return _NC_CACHE["nc"]


def _f16_rtz(a: np.ndarray) -> np.ndarray:
    """Round-toward-zero f32 -> f16 (exact for the predicate `< 1`)."""
    v = np.ascontiguousarray(a, dtype=np.float32).view(np.uint32)
    return (v & np.uint32(0xFFFFE000)).view(np.float32).astype(np.float16)


def _f16_rtp(a: np.ndarray) -> np.ndarray:
    """Round-toward-+inf f32 -> f16 (exact for the predicate `> 0.5`)."""
    v = np.ascontiguousarray(a, dtype=np.float32).view(np.uint32)
    frac = v & np.uint32(0x1FFF)
    t = (v & ~np.uint32(0x1FFF)) + np.where(frac != 0, np.uint32(0x2000),
                                            np.uint32(0))
    return t.view(np.float32).astype(np.float16)


def _permute_w(W: np.ndarray) -> np.ndarray:
    # [IN_DIM, N_RULES] -> [KP*NB, P, 2*NB_W] f16: for k-pair kk, n-block
    # nb, row p holds [W[2kk*128+p, block], W[(2kk+1)*128+p, block]]
    w5 = _f16_rtp(W).reshape(KP, 2, P, NB, NB_W)     # [kk, j, p, nb, n]
    return np.ascontiguousarray(
        w5.transpose(0, 3, 2, 1, 4).reshape(KP * NB, P, 2 * NB_W))


def _permute_x(x_shard: np.ndarray) -> np.ndarray:
    # [M_LOCAL, IN_DIM] -> [KP, P, 2*M_LOCAL] f16: row p of slab kk holds
    # [x[:, 2kk*128+p].T, x[:, (2kk+1)*128+p].T]
    x4 = _f16_rtz(x_shard).T.reshape(KP, 2, P, M_LOCAL)  # [kk, j, p, m]
    return np.ascontiguousarray(x4.transpose(0, 2, 1, 3).reshape(
        KP, P, 2 * M_LOCAL))


def kernel(x: np.ndarray, W: np.ndarray, **run_kwargs) -> np.ndarray:
    assert x.shape == (BATCH, IN_DIM) and W.shape == (IN_DIM, N_RULES)
    nc = _get_nc()
    wp = _permute_w(W)
    in_maps = []
    for c in range(N_CORES):
        in_maps.append({"xp": _permute_x(x[c * M_LOCAL:(c + 1) * M_LOCAL, :]),
                        "wp": wp})
    res = run_bass_kernel_spmd(nc, in_maps, core_ids=list(range(N_CORES)),
                               **run_kwargs)
    out = np.concatenate([res.results[c]["out"] for c in range(N_CORES)],
                         axis=0).astype(np.float32)  # bf16 {0,1} -> f32 exact
    if run_kwargs:
        kernel.last_results = res
    return out
